# revision 9
# baseline (speedup 1.0000x reference)
"""Bass/Trainium2 kernel for nn_ContrastiveLoss (8-core SPMD).

Math (matching the reference):
    S_xy = exp(x @ yf.T / TEMP)   [N, T*Q]   yf = y.reshape(T*Q, d)
    S_xx = exp(x @ x.T / TEMP)    [N, N]
    per-row scalars:
      rxy_total[n] = sum_m S_xy[n, m]
      rxy_pos[n]   = sum_m S_xy[n, m] * (m % T == tid[n])
      rxx_total[n] = sum_m S_xx[n, m]
      rxx_posf[n]  = sum_m S_xx[n, m] * (tid[m] == tid[n])
    num_i = rxy_pos + 0.5*(rxx_posf - diag)     diag = exp(||x_n||^2/TEMP)
    den_i = (rxy_total - rxy_pos) + (rxx_total - rxx_posf)
    then a T-sized segment sum over tid and the log-ratio mean (host).

Sharding over 8 cores:
  - xy stage: column-parallel. Core c owns yf rows [c*2048, (c+1)*2048) and
    computes partial rxy_total / rxy_pos for ALL 2048 x-rows; partials are
    summed on the host. (m % T) for core c's local column k is k % T since
    2048 is a multiple of T=256.
  - xx stage: row-parallel. Core c owns x rows [c*256, (c+1)*256) and
    computes full rxx_total / rxx_posf for those rows against all of x.

Device kernel per core:
  - fp8 e4m3 DoubleRow matmuls (inputs prescaled by 64 on the host), one
    matmul covers the full K=256 contraction via [128, 2, F] operands
  - exp on ScalarE straight out of PSUM (xx rows also take row totals via
    accum_out; xy row totals instead come from a reduce over the fold tree
    to keep ScalarE's accumulator drains off the critical path)
  - fold tree s(2048) -> g3(512) split between GpSimd (g1) and Vector
    (g2, g3); "positive"/total sums as two fused mul-reduce ops on Vector
  - inputs land as a handful of large p-major contiguous DMAs (the DMA
    sequencer is descriptor-generation-bound on small strided runs)
"""

import numpy as np
from contextlib import ExitStack

import concourse.bass as bass
import concourse.bacc as bacc
import concourse.mybir as mybir
import concourse.tile as tile
from concourse import bass_utils

N, D, T, Q = 2048, 256, 256, 64
TEMP = 0.3
NCORES = 8
R = N // NCORES            # 256 x-rows per core (xx stage)
YC = (T * Q) // NCORES     # 2048 y-cols per core (xy stage)
NRG = N // 128             # 16 row groups (xy stage)
NXG = R // 128             # 2 row groups (xx stage)
NB = 512                   # matmul moving-operand block (one PSUM bank)
NA = 512                   # head chunk landed before the big streaming DMAs
SQ = 64.0                  # fp8 prescale; matmul result is SQ^2 * dot
QS = 1.0 / (SQ * SQ * TEMP)

F32 = mybir.dt.float32
BF16 = mybir.dt.bfloat16
F8 = mybir.dt.float8e4
AF = mybir.ActivationFunctionType
ALU = mybir.AluOpType
DR = mybir.MatmulPerfMode.DoubleRow


def _build_bass():
    nc = bacc.Bacc(
        "TRN2",
        target_bir_lowering=False,
        debug=False,
        enable_asserts=False,
        num_devices=NCORES,
    )
    xta = nc.dram_tensor("xta", [128, 2, NA], F8, kind="ExternalInput").ap()
    xtb = nc.dram_tensor("xtb", [128, 2, N - NA], F8, kind="ExternalInput").ap()
    yta = nc.dram_tensor("yta", [128, NA, 2], F8, kind="ExternalInput").ap()
    ytb = nc.dram_tensor("ytb", [128, YC - NA, 2], F8, kind="ExternalInput").ap()
    xtl = nc.dram_tensor("xtl", [128, 2, R], F8, kind="ExternalInput").ap()
    oh = nc.dram_tensor("oh", [128, NRG, T], F8, kind="ExternalInput").ap()
    mmk = nc.dram_tensor("mmk", [128, NXG, N], F8, kind="ExternalInput").ap()
    out = nc.dram_tensor("out", [128, 2 * NRG + 2 * NXG], F32, kind="ExternalOutput").ap()

    with tile.TileContext(nc) as tc:
        _kernel(tc, out, xta, xtb, yta, ytb, xtl, oh, mmk)
    nc.compile()
    _dedup_ldweights(nc)
    return nc


def _dedup_ldweights(nc):
    """Drop InstLdweights that reload the weights already in the PE array.

    After compile each matmul is paired with its own LDWEIGHTS even when four
    consecutive matmuls share one stationary tile. Keep the first, remove
    duplicates that carry no semaphore waits/updates (removing those would
    break the schedule's sem arithmetic).
    """
    pe_engines = {mybir.EngineType.PE}
    removed = 0
    for blk in nc.m.functions[0].blocks:
        last_w = None
        keep = []
        for i in blk.instructions:
            t = type(i).__name__
            eng = getattr(i, "engine", None)
            if eng in pe_engines:
                if t == "InstLdweights":
                    w = str(i.ins[0].concise())
                    si = i.sync_info
                    clean = si is None or (not si.on_wait and not si.on_update)
                    if w == last_w and clean:
                        removed += 1
                        continue
                    last_w = w
                elif t != "InstMatmult":
                    last_w = None
            keep.append(i)
        blk.instructions[:] = keep
    return removed


def _kernel(tc, out, xta, xtb, yta, ytb, xtl, oh, mmk):
    nc = tc.nc
    with ExitStack() as ctx:
        const = ctx.enter_context(tc.tile_pool(name="const", bufs=1))
        psum = ctx.enter_context(tc.tile_pool(name="psum", bufs=2, space="PSUM"))
        sp = ctx.enter_context(tc.tile_pool(name="sp", bufs=3))
        scrp = ctx.enter_context(tc.tile_pool(name="scrp", bufs=3))

        # Contiguous p-major tiles; the A/B split lets rg0's operands land
        # first while keeping every DMA a full-tensor contiguous transfer.
        xta_sb = const.tile([128, 2, NA], F8, name="xta_sb")
        yta_sb = const.tile([128, NA, 2], F8, name="yta_sb")
        xtb_sb = const.tile([128, 2, N - NA], F8, name="xtb_sb")
        ytb_sb = const.tile([128, YC - NA, 2], F8, name="ytb_sb")
        oh_sb = const.tile([128, NRG, T], F8, name="oh_sb")
        mm_sb = const.tile([128, NXG, N], F8, name="mm_sb")
        xtl_sb = const.tile([128, 2, R], F8, name="xtl_sb")
        nc.sync.dma_start(out=xta_sb, in_=xta)
        nc.sync.dma_start(out=yta_sb, in_=yta)
        nc.sync.dma_start(out=ytb_sb, in_=ytb)
        nc.sync.dma_start(out=oh_sb, in_=oh)
        nc.sync.dma_start(out=xtb_sb, in_=xtb)
        nc.sync.dma_start(out=mm_sb, in_=mmk)
        nc.sync.dma_start(out=xtl_sb, in_=xtl)

        acc = const.tile([128, 2 * NRG + 2 * NXG], F32, name="acc")
        pos_acc = acc[:, 0:NRG]
        tot_acc = acc[:, NRG:2 * NRG]
        xxp_acc = acc[:, 2 * NRG:2 * NRG + NXG]
        xxt_acc = acc[:, 2 * NRG + NXG:2 * NRG + 2 * NXG]

        H = YC // 2   # 1024: fold half
        Qk = YC // 4  # 512: fold quarter

        def x_sl(rg):
            if rg * 128 < NA:
                return xta_sb[:, :, rg * 128:(rg + 1) * 128]
            return xtb_sb[:, :, rg * 128 - NA:(rg + 1) * 128 - NA]

        def y_mv(cb):
            if cb * NB < NA:
                t = yta_sb[:, cb * NB:(cb + 1) * NB, :]
            else:
                t = ytb_sb[:, cb * NB - NA:(cb + 1) * NB - NA, :]
            return t.rearrange("p c j -> p j c")

        def x_mv(cb):
            if cb * NB < NA:
                return xta_sb[:, :, cb * NB:(cb + 1) * NB]
            return xtb_sb[:, :, cb * NB - NA:(cb + 1) * NB - NA]

        def xy_rg(rg):
            pt = psum.tile([128, YC], F32, name="pt_xy", tag="pt")
            lhsT = x_sl(rg)
            for cb in range(YC // NB):
                nc.tensor.matmul(
                    pt[:, cb * NB:(cb + 1) * NB],
                    lhsT,
                    y_mv(cb),
                    start=True,
                    stop=True,
                    perf_mode=DR,
                )
            # fold engine: idle SDMA engines (accumulating copies) for the
            # early row-groups, GpSimd g1 for the middle ones, all-Vector for
            # the tail (totals ride ScalarE's accumulator there)
            mode = "gps" if rg < 13 else "v"
            s = sp.tile([128, YC], BF16, name="s_xy", tag="s")
            nc.scalar.activation(
                out=s, in_=pt, func=AF.Exp, scale=QS,
                accum_out=tot_acc[:, rg:rg + 1],
            )
            g3 = scrp.tile([128, Qk], BF16, name="g3_xy", tag="g3")
            g1 = scrp.tile([128, Qk], BF16, name="g1_xy", tag="g1")
            eng1 = nc.gpsimd if mode == "gps" else nc.vector
            eng1.tensor_tensor(
                out=g1, in0=s[:, 0:Qk], in1=s[:, Qk:H], op=ALU.add)
            g2 = scrp.tile([128, Qk], BF16, name="g2_xy", tag="g2")
            nc.vector.tensor_tensor(
                out=g2, in0=s[:, H:H + Qk], in1=s[:, H + Qk:YC], op=ALU.add)
            nc.vector.tensor_tensor(out=g3, in0=g1, in1=g2, op=ALU.add)
            scr = scrp.tile([128, Qk], BF16, name="scr_xy", tag="scr")
            nc.vector.affine_mul_reduce(
                out=scr.rearrange("p (j t) -> p j t", t=T),
                accum_out=pos_acc[:, rg:rg + 1],
                in0=g3.rearrange("p (j t) -> p j t", t=T),
                in1=oh_sb[:, rg, :].unsqueeze(1).broadcast_to((128, Qk // T, T)),
                scale=1.0,
                bias=0.0,
            )

        # ---- xy stage: all x rows vs this core's y columns ----
        for rg in range(8):
            xy_rg(rg)

        # ---- xx stage: this core's x rows vs all x columns ----
        for g in range(NXG):
            pt = psum.tile([128, N], F32, name="pt_xx", tag="pt")
            lhsT = xtl_sb[:, :, g * 128:(g + 1) * 128]
            for cb in range(N // NB):
                nc.tensor.matmul(
                    pt[:, cb * NB:(cb + 1) * NB],
                    lhsT,
                    x_mv(cb),
                    start=True,
                    stop=True,
                    perf_mode=DR,
                )
            s = sp.tile([128, N], BF16, name="s_xx", tag="s")
            nc.scalar.activation(
                out=s, in_=pt, func=AF.Exp, scale=QS,
                accum_out=xxt_acc[:, g:g + 1],
            )
            scr = scrp.tile([128, N], F32, name="scr_xx", tag="scrxx")
            nc.vector.affine_mul_reduce(
                out=scr,
                accum_out=xxp_acc[:, g:g + 1],
                in0=s,
                in1=mm_sb[:, g, :],
                scale=1.0,
                bias=0.0,
            )

        for rg in range(8, NRG):
            xy_rg(rg)

        nc.sync.dma_start(out=out, in_=acc)


def _pack_kT(a):
    """[F, D] f32 -> [128, 2, F] fp8 with [p, j, f] = q(a[f, 128j+p] * SQ)."""
    import ml_dtypes
    q = (a.T * SQ).astype(ml_dtypes.float8_e4m3)      # [D, F]
    return np.ascontiguousarray(q.reshape(2, 128, -1).transpose(1, 0, 2))


def make_in_maps(x, tid):
    """Per-core input dicts. x: [N, D] f32; tid: [N] int."""
    import ml_dtypes
    xq = _pack_kT(x)                                   # [128, 2, N]
    oh = np.zeros((N, T), ml_dtypes.float8_e4m3)
    oh[np.arange(N), tid] = 1.0
    # device oh_sb[p, rg, t] pairs partition p with x row 128*rg + p
    oh3 = np.ascontiguousarray(oh.reshape(NRG, 128, T).transpose(1, 0, 2))

    in_maps = []
    for c in range(NCORES):
        xtl = np.ascontiguousarray(xq[:, :, c * R:(c + 1) * R])
        rows = tid[c * R:(c + 1) * R]
        mm = (rows[:, None] == tid[None, :]).astype(ml_dtypes.float8_e4m3)
        mm3 = np.ascontiguousarray(mm.reshape(NXG, 128, N).transpose(1, 0, 2))
        in_maps.append({
            "xta": np.ascontiguousarray(xq[:, :, 0:NA]),
            "xtb": np.ascontiguousarray(xq[:, :, NA:]),
            "xtl": xtl,
            "yta": None,  # filled below (depends on y)
            "ytb": None,
            "oh": oh3,
            "mmk": mm3,
        })
    return in_maps


def fill_y_slices(in_maps, y):
    yf = np.ascontiguousarray(y.reshape(T * Q, D))
    for c in range(NCORES):
        ys = yf[c * YC:(c + 1) * YC]          # [YC, D]
        yq = _pack_kT(ys)                      # [128, 2, YC]
        yqi = yq.transpose(0, 2, 1)            # [128, YC, 2] pairs adjacent
        in_maps[c]["yta"] = np.ascontiguousarray(yqi[:, 0:NA, :])
        in_maps[c]["ytb"] = np.ascontiguousarray(yqi[:, NA:, :])


def combine(outs, x, tid):
    """outs: list of per-core 'out' arrays [128, 36]. Returns loss [1] f32."""
    import ml_dtypes
    rxy_pos = np.zeros(N, np.float64)
    rxy_tot = np.zeros(N, np.float64)
    rxx_posf = np.zeros(N, np.float64)
    rxx_tot = np.zeros(N, np.float64)
    for c, o in enumerate(outs):
        o = o.astype(np.float64)
        # xy partials cover all rows; row of (p, rg) is 128*rg + p
        rxy_pos += o[:, 0:NRG].T.reshape(N)
        rxy_tot += o[:, NRG:2 * NRG].T.reshape(N)
        # xx covers this core's rows only
        rxx_posf[c * R:(c + 1) * R] = o[:, 2 * NRG:2 * NRG + NXG].T.reshape(R)
        rxx_tot[c * R:(c + 1) * R] = o[:, 2 * NRG + NXG:2 * NRG + 2 * NXG].T.reshape(R)

    # diagonal of S_xx as the device computed it: from the fp8-quantized x
    xq = (x.astype(np.float64) * SQ).astype(ml_dtypes.float8_e4m3).astype(np.float64)
    diag = np.exp((xq * xq).sum(axis=1) / (SQ * SQ * TEMP))

    num_i = rxy_pos + 0.5 * (rxx_posf - diag)
    den_i = (rxy_tot - rxy_pos) + (rxx_tot - rxx_posf)

    num_t = np.bincount(tid, weights=num_i, minlength=T)
    den_t = np.bincount(tid, weights=den_i, minlength=T)
    counts = np.bincount(tid, minlength=T)
    present = counts > 0
    loss_t = -np.log(num_t[present] / (den_t[present] + num_t[present]))
    loss = loss_t.sum() / present.sum()
    return np.asarray([loss], dtype=np.float32)


_NC_CACHE = None


def _get_nc():
    global _NC_CACHE
    if _NC_CACHE is None:
        _NC_CACHE = _build_bass()
    return _NC_CACHE


def kernel(x, track_idxs, y, _trace=False):
    x = np.ascontiguousarray(np.asarray(x), dtype=np.float32)
    y = np.ascontiguousarray(np.asarray(y), dtype=np.float32)
    tid = np.asarray(track_idxs).astype(np.int64)

    nc = _get_nc()
    in_maps = make_in_maps(x, tid)
    fill_y_slices(in_maps, y)

    res = bass_utils.run_bass_kernel_spmd(
        nc, in_maps, core_ids=list(range(NCORES)), trace=_trace,
    )
    outs = [r["out"] for r in res.results]
    loss = combine(outs, x, tid)
    if _trace:
        return loss, res
    return loss


# revision 10
# speedup vs baseline: 1.1897x; 1.1897x over previous
"""Bass/Trainium2 kernel for nn_ContrastiveLoss (8-core SPMD).

Math (matching the reference):
    S_xy = exp(x @ yf.T / TEMP)   [N, T*Q]   yf = y.reshape(T*Q, d)
    S_xx = exp(x @ x.T / TEMP)    [N, N]
    per-row scalars:
      rxy_total[n] = sum_m S_xy[n, m]
      rxy_pos[n]   = sum_m S_xy[n, m] * (m % T == tid[n])
      rxx_total[n] = sum_m S_xx[n, m]
      rxx_posf[n]  = sum_m S_xx[n, m] * (tid[m] == tid[n])
    num_i = rxy_pos + 0.5*(rxx_posf - diag)     diag = exp(||x_n||^2/TEMP)
    den_i = (rxy_total - rxy_pos) + (rxx_total - rxx_posf)
    then a T-sized segment sum over tid and the log-ratio mean (host).

Sharding over 8 cores:
  - xy stage: column-parallel. Core c owns yf rows [c*2048, (c+1)*2048) and
    computes partial rxy_total / rxy_pos for ALL 2048 x-rows; partials are
    summed on the host.
  - xx stage: row-parallel. Core c owns 256 x rows and computes full
    rxx_total / rxx_posf for those rows against all of x.

Key tricks:
  - Rows are SORTED by track id on the host (the loss is invariant to row
    order). Each 128-row group then spans ~18 consecutive tracks, so the
    one-hot "positive" reduce runs on a static 64-track window instead of
    all 256 (saving VectorE time), and each core's same-track x-columns fit
    a 1024-column window of a per-core column-rotated copy of x^T.
  - fp8 e4m3 matmuls (inputs prescaled by 64), DoubleRow packs K=256 into
    one matmul via [128, 2, F] operands.
  - exp on ScalarE straight from PSUM; xy row totals ride the fused
    scalar_tensor_tensor (last fold + accumulate) on VectorE so ScalarE
    pays no accumulator drains for the 16 xy activations.
  - inputs land as a few large p-major contiguous DMAs (the DMA sequencer
    is descriptor-generation-bound on small strided runs).
"""

import numpy as np
from contextlib import ExitStack

import concourse.bass as bass
import concourse.bacc as bacc
import concourse.mybir as mybir
import concourse.tile as tile
from concourse import bass_utils

N, D, T, Q = 2048, 256, 256, 64
TEMP = 0.3
NCORES = 8
R = N // NCORES            # 256 x-rows per core (xx stage)
YC = (T * Q) // NCORES     # 2048 y-cols per core (xy stage)
NRG = N // 128             # 16 row groups (xy stage)
NXG = R // 128             # 2 row groups (xx stage)
NB = 512                   # matmul moving-operand block (one PSUM bank)
NA = 512                   # head chunk landed before the big streaming DMAs
BW = 64                    # xy positive-mask track window (sorted rows)
XW = 1024                  # xx positive-mask column window (rotated cols)
XSH = 384                  # xx window: rolled col k = global col 256c-XSH+k
SQ = 64.0                  # fp8 prescale; matmul result is SQ^2 * dot
QS = 1.0 / (SQ * SQ * TEMP)

F32 = mybir.dt.float32
BF16 = mybir.dt.bfloat16
F8 = mybir.dt.float8e4
AF = mybir.ActivationFunctionType
ALU = mybir.AluOpType
DR = mybir.MatmulPerfMode.DoubleRow


def _win0(rg):
    return min(max(16 * rg - 16, 0), T - BW)


def _build_bass():
    nc = bacc.Bacc(
        "TRN2",
        target_bir_lowering=False,
        debug=False,
        enable_asserts=False,
        num_devices=NCORES,
    )
    xta = nc.dram_tensor("xta", [128, 2, NA], F8, kind="ExternalInput").ap()
    xtb = nc.dram_tensor("xtb", [128, 2, N - NA], F8, kind="ExternalInput").ap()
    yta = nc.dram_tensor("yta", [128, 2, NA], F8, kind="ExternalInput").ap()
    ytb = nc.dram_tensor("ytb", [128, 2, YC - NA], F8, kind="ExternalInput").ap()
    xtr = nc.dram_tensor("xtr", [128, 2, N], F8, kind="ExternalInput").ap()
    xtl = nc.dram_tensor("xtl", [128, 2, R], F8, kind="ExternalInput").ap()
    oh = nc.dram_tensor("oh", [128, NRG, BW], F8, kind="ExternalInput").ap()
    mmk = nc.dram_tensor("mmk", [128, NXG, XW], F8, kind="ExternalInput").ap()
    out = nc.dram_tensor("out", [128, 2 * NRG + 2 * NXG], F32, kind="ExternalOutput").ap()

    with tile.TileContext(nc) as tc:
        _kernel(tc, out, xta, xtb, yta, ytb, xtr, xtl, oh, mmk)
    nc.compile()
    _dedup_ldweights(nc)
    return nc


def _dedup_ldweights(nc):
    """Drop InstLdweights that reload the weights already in the PE array."""
    pe_engines = {mybir.EngineType.PE}
    removed = 0
    for blk in nc.m.functions[0].blocks:
        last_w = None
        keep = []
        for i in blk.instructions:
            t = type(i).__name__
            eng = getattr(i, "engine", None)
            if eng in pe_engines:
                if t == "InstLdweights":
                    w = str(i.ins[0].concise())
                    si = i.sync_info
                    clean = si is None or (not si.on_wait and not si.on_update)
                    if w == last_w and clean:
                        removed += 1
                        continue
                    last_w = w
                elif t != "InstMatmult":
                    last_w = None
            keep.append(i)
        blk.instructions[:] = keep
    return removed


def _kernel(tc, out, xta, xtb, yta, ytb, xtr, xtl, oh, mmk):
    nc = tc.nc
    with ExitStack() as ctx:
        const = ctx.enter_context(tc.tile_pool(name="const", bufs=1))
        psum = ctx.enter_context(tc.tile_pool(name="psum", bufs=2, space="PSUM"))
        sp = ctx.enter_context(tc.tile_pool(name="sp", bufs=3))
        scrp = ctx.enter_context(tc.tile_pool(name="scrp", bufs=3))

        xta_sb = const.tile([128, 2, NA], F8, name="xta_sb")
        yta_sb = const.tile([128, 2, NA], F8, name="yta_sb")
        xtb_sb = const.tile([128, 2, N - NA], F8, name="xtb_sb")
        ytb_sb = const.tile([128, 2, YC - NA], F8, name="ytb_sb")
        xtr_sb = const.tile([128, 2, N], F8, name="xtr_sb")
        oh_sb = const.tile([128, NRG, BW], F8, name="oh_sb")
        mm_sb = const.tile([128, NXG, XW], F8, name="mm_sb")
        xtl_sb = const.tile([128, 2, R], F8, name="xtl_sb")
        nc.sync.dma_start(out=xta_sb, in_=xta)
        nc.sync.dma_start(out=yta_sb, in_=yta)
        nc.sync.dma_start(out=ytb_sb, in_=ytb)
        nc.sync.dma_start(out=oh_sb, in_=oh)
        nc.sync.dma_start(out=xtb_sb, in_=xtb)
        nc.sync.dma_start(out=xtr_sb, in_=xtr)
        nc.sync.dma_start(out=mm_sb, in_=mmk)
        nc.sync.dma_start(out=xtl_sb, in_=xtl)

        acc = const.tile([128, 2 * NRG + 2 * NXG], F32, name="acc")
        pos_acc = acc[:, 0:NRG]
        tot_acc = acc[:, NRG:2 * NRG]
        xxp_acc = acc[:, 2 * NRG:2 * NRG + NXG]
        xxt_acc = acc[:, 2 * NRG + NXG:2 * NRG + 2 * NXG]

        H = YC // 2   # 1024: fold half
        Qk = YC // 4  # 512: fold quarter

        def x_sl(rg):
            if rg * 128 < NA:
                return xta_sb[:, :, rg * 128:(rg + 1) * 128]
            return xtb_sb[:, :, rg * 128 - NA:(rg + 1) * 128 - NA]

        def y_mv(cb):
            if cb * NB < NA:
                return yta_sb[:, :, cb * NB:(cb + 1) * NB]
            return ytb_sb[:, :, cb * NB - NA:(cb + 1) * NB - NA]

        def xy_rg(rg):
            pt = psum.tile([128, YC], F32, name="pt_xy", tag="pt")
            lhsT = x_sl(rg)
            for cb in range(YC // NB):
                nc.tensor.matmul(
                    pt[:, cb * NB:(cb + 1) * NB],
                    lhsT,
                    y_mv(cb),
                    start=True,
                    stop=True,
                    perf_mode=DR,
                )
            s = sp.tile([128, YC], BF16, name="s_xy", tag="s")
            nc.scalar.activation(out=s, in_=pt, func=AF.Exp, scale=QS)
            g1 = scrp.tile([128, Qk], BF16, name="g1_xy", tag="g1")
            nc.vector.tensor_tensor(
                out=g1, in0=s[:, 0:Qk], in1=s[:, Qk:H], op=ALU.add)
            g2 = scrp.tile([128, Qk], BF16, name="g2_xy", tag="g2")
            nc.vector.tensor_tensor(
                out=g2, in0=s[:, H:H + Qk], in1=s[:, H + Qk:YC], op=ALU.add)
            # fused last fold + row total: g3 = g1 + g2, tot = sum(g3)
            g3 = scrp.tile([128, Qk], BF16, name="g3_xy", tag="g3")
            nc.vector.scalar_tensor_tensor(
                out=g3, in0=g1, scalar=1.0, in1=g2,
                op0=ALU.mult, op1=ALU.add,
                accum_out=tot_acc[:, rg:rg + 1],
            )
            # positive mask reduce on the static 64-track window of this
            # row-group (rows are track-sorted on the host)
            c0 = _win0(rg)
            scr = scrp.tile([128, 2 * BW], BF16, name="scr_xy", tag="scr")
            nc.vector.affine_mul_reduce(
                out=scr.rearrange("p (j t) -> p j t", t=BW),
                accum_out=pos_acc[:, rg:rg + 1],
                in0=g3.rearrange("p (j t) -> p j t", t=T)[:, :, c0:c0 + BW],
                in1=oh_sb[:, rg, :].unsqueeze(1).broadcast_to((128, Qk // T, BW)),
                scale=1.0,
                bias=0.0,
            )

        # ---- xy stage: all x rows vs this core's y columns ----
        for rg in range(8):
            xy_rg(rg)

        # ---- xx stage: this core's x rows vs all x (rotated) columns ----
        for g in range(NXG):
            pt = psum.tile([128, N], F32, name="pt_xx", tag="pt")
            lhsT = xtl_sb[:, :, g * 128:(g + 1) * 128]
            for cb in range(N // NB):
                nc.tensor.matmul(
                    pt[:, cb * NB:(cb + 1) * NB],
                    lhsT,
                    xtr_sb[:, :, cb * NB:(cb + 1) * NB],
                    start=True,
                    stop=True,
                    perf_mode=DR,
                )
            s = sp.tile([128, N], BF16, name="s_xx", tag="s")
            nc.scalar.activation(
                out=s, in_=pt, func=AF.Exp, scale=QS,
                accum_out=xxt_acc[:, g:g + 1],
            )
            scr = scrp.tile([128, XW], F32, name="scr_xx", tag="scrxx")
            nc.vector.affine_mul_reduce(
                out=scr,
                accum_out=xxp_acc[:, g:g + 1],
                in0=s[:, 0:XW],
                in1=mm_sb[:, g, :],
                scale=1.0,
                bias=0.0,
            )

        for rg in range(8, NRG):
            xy_rg(rg)

        nc.sync.dma_start(out=out, in_=acc)


def _pack_kT(a):
    """[F, D] f32 -> [128, 2, F] fp8 with [p, j, f] = q(a[f, 128j+p] * SQ)."""
    import ml_dtypes
    q = (a.T * SQ).astype(ml_dtypes.float8_e4m3)      # [D, F]
    return np.ascontiguousarray(q.reshape(2, 128, -1).transpose(1, 0, 2))


def _windows_ok(tids):
    """Check the sorted track ids fit the compiled mask windows."""
    for rg in range(NRG):
        c0 = _win0(rg)
        t = tids[rg * 128:(rg + 1) * 128]
        if t.min() < c0 or t.max() >= c0 + BW:
            return False
    for c in range(NCORES):
        a, b = tids[c * R], tids[(c + 1) * R - 1]
        lo = np.searchsorted(tids, a, side="left")
        hi = np.searchsorted(tids, b, side="right")
        if lo < c * R - XSH or hi > c * R - XSH + XW:
            return False
    return True


def make_in_maps(xs, tids):
    """Per-core input dicts. xs: sorted [N, D] f32; tids: sorted [N] int."""
    import ml_dtypes
    xq = _pack_kT(xs)                                  # [128, 2, N]
    ohn = np.zeros((NRG, 128, BW), ml_dtypes.float8_e4m3)
    for rg in range(NRG):
        c0 = _win0(rg)
        ohn[rg, np.arange(128), tids[rg * 128:(rg + 1) * 128] - c0] = 1.0
    oh3 = np.ascontiguousarray(ohn.transpose(1, 0, 2))  # [128, NRG, BW]

    in_maps = []
    for c in range(NCORES):
        xtl = np.ascontiguousarray(xq[:, :, c * R:(c + 1) * R])
        # per-core column rotation: rolled col k = global col (c*R - XSH + k)
        sh = c * R - XSH
        gcols = (np.arange(N) + sh) % N
        xtr = np.ascontiguousarray(xq[:, :, gcols])
        rows = tids[c * R:(c + 1) * R]
        mm = (rows[:, None] == tids[gcols[:XW]][None, :]).astype(
            ml_dtypes.float8_e4m3)
        mm3 = np.ascontiguousarray(mm.reshape(NXG, 128, XW).transpose(1, 0, 2))
        in_maps.append({
            "xta": np.ascontiguousarray(xq[:, :, 0:NA]),
            "xtb": np.ascontiguousarray(xq[:, :, NA:]),
            "xtr": xtr,
            "xtl": xtl,
            "yta": None,  # filled below (depends on y)
            "ytb": None,
            "oh": oh3,
            "mmk": mm3,
        })
    return in_maps


def fill_y_slices(in_maps, y):
    yf = np.ascontiguousarray(y.reshape(T * Q, D))
    for c in range(NCORES):
        ys = yf[c * YC:(c + 1) * YC]          # [YC, D]
        yq = _pack_kT(ys)                      # [128, 2, YC]
        in_maps[c]["yta"] = np.ascontiguousarray(yq[:, :, 0:NA])
        in_maps[c]["ytb"] = np.ascontiguousarray(yq[:, :, NA:])


def combine(outs, xs, tids):
    """outs: per-core [128, 36] accs (sorted-row order). Returns loss [1]."""
    import ml_dtypes
    rxy_pos = np.zeros(N, np.float64)
    rxy_tot = np.zeros(N, np.float64)
    rxx_posf = np.zeros(N, np.float64)
    rxx_tot = np.zeros(N, np.float64)
    for c, o in enumerate(outs):
        o = o.astype(np.float64)
        rxy_pos += o[:, 0:NRG].T.reshape(N)
        rxy_tot += o[:, NRG:2 * NRG].T.reshape(N)
        rxx_posf[c * R:(c + 1) * R] = o[:, 2 * NRG:2 * NRG + NXG].T.reshape(R)
        rxx_tot[c * R:(c + 1) * R] = o[:, 2 * NRG + NXG:2 * NRG + 2 * NXG].T.reshape(R)

    # diagonal of S_xx as the device computed it: from the fp8-quantized x
    xq = (xs.astype(np.float64) * SQ).astype(ml_dtypes.float8_e4m3).astype(np.float64)
    diag = np.exp((xq * xq).sum(axis=1) / (SQ * SQ * TEMP))

    num_i = rxy_pos + 0.5 * (rxx_posf - diag)
    den_i = (rxy_tot - rxy_pos) + (rxx_tot - rxx_posf)

    num_t = np.bincount(tids, weights=num_i, minlength=T)
    den_t = np.bincount(tids, weights=den_i, minlength=T)
    counts = np.bincount(tids, minlength=T)
    present = counts > 0
    loss_t = -np.log(num_t[present] / (den_t[present] + num_t[present]))
    loss = loss_t.sum() / present.sum()
    return np.asarray([loss], dtype=np.float32)


def _host_reference(x, tid, y):
    """Exact loss on the host; fallback if mask windows don't hold."""
    x64 = x.astype(np.float64)
    yf = y.reshape(T * Q, D).astype(np.float64)
    sxy = np.exp(x64 @ yf.T / TEMP)
    sxx = np.exp(x64 @ x64.T / TEMP)
    y_idx = np.tile(np.arange(T), Q)
    oh_y = (y_idx[None, :] == tid[:, None])
    oh_x = (tid[None, :] == tid[:, None])
    rxy_pos = (sxy * oh_y).sum(1)
    rxy_tot = sxy.sum(1)
    rxx_posf = (sxx * oh_x).sum(1)
    rxx_tot = sxx.sum(1)
    num_i = rxy_pos + 0.5 * (rxx_posf - np.diagonal(sxx))
    den_i = (rxy_tot - rxy_pos) + (rxx_tot - rxx_posf)
    num_t = np.bincount(tid, weights=num_i, minlength=T)
    den_t = np.bincount(tid, weights=den_i, minlength=T)
    present = np.bincount(tid, minlength=T) > 0
    loss_t = -np.log(num_t[present] / (den_t[present] + num_t[present]))
    return np.asarray([loss_t.sum() / present.sum()], dtype=np.float32)


_NC_CACHE = None


def _get_nc():
    global _NC_CACHE
    if _NC_CACHE is None:
        _NC_CACHE = _build_bass()
    return _NC_CACHE


def kernel(x, track_idxs, y, _trace=False):
    x = np.ascontiguousarray(np.asarray(x), dtype=np.float32)
    y = np.ascontiguousarray(np.asarray(y), dtype=np.float32)
    tid = np.asarray(track_idxs).astype(np.int64)

    # sort rows by track id: the loss is invariant, and sorted rows make the
    # positive-pair masks (nearly) block-diagonal so the kernel only reduces
    # narrow static windows
    perm = np.argsort(tid, kind="stable")
    xs = np.ascontiguousarray(x[perm])
    tids = tid[perm]

    if not _windows_ok(tids):
        loss = _host_reference(x, tid, y)
        return (loss, None) if _trace else loss

    nc = _get_nc()
    in_maps = make_in_maps(xs, tids)
    fill_y_slices(in_maps, y)

    res = bass_utils.run_bass_kernel_spmd(
        nc, in_maps, core_ids=list(range(NCORES)), trace=_trace,
    )
    outs = [r["out"] for r in res.results]
    loss = combine(outs, xs, tids)
    if _trace:
        return loss, res
    return loss
